# revision 2
# baseline (speedup 1.0000x reference)
"""GNN ensemble MoE-routing kernel for Trainium2 (8 NeuronCores).

Reference computes all 8 expert MLPs for every sample then selects one
(8x wasted FLOPs). This kernel routes on the host instead: samples are
gathered per expert, and core c runs ONLY expert c's MLP over the
samples routed to it (expert-parallel sharding).

Math folding (exact):
  lat = eps*sigma_c + mu_c  =>  lat @ W1_c = eps @ (sigma_c*W1_c) + mu_c@W1_c
so the device computes  sigmoid(eps @ W1p + b1p) @ W2 + b2  with
  W1p = sigma_c * W1_c,  b1p = b1_c + mu_c @ W1_c  (folded on host).

Device layout: features on SBUF partitions, samples on the free axis.
  epsT [512, K]  ->  H^T [1024, K]  ->  Y^T [512, K]
Weights load in natural layout as matmul lhsT (out = lhsT.T @ rhs), the
b1 bias+sigmoid fuse into one ScalarE activation (per-partition bias),
b2 is a DVE tensor_scalar add. Matmuls run as float32r (1 cycle/row for
moving dim >= 256, vs 4 cycles/row for plain fp32).
"""

from contextlib import ExitStack

import numpy as np

import concourse.bass as bass
import concourse.tile as tile
from concourse import bacc, mybir
from concourse.bass_utils import run_bass_kernel_spmd

NB_COMP = 8
LAT_DIM = 512
NB_NEUR = 1024
OUT_DIM = 512
N_CORES = 8

F32 = mybir.dt.float32
F32R = mybir.dt.float32r
SIG = mybir.ActivationFunctionType.Sigmoid

_program_cache = {}


def _make_chunks(k_cap):
    chunks = []
    n0 = 0
    while n0 < k_cap:
        ns = min(512, k_cap - n0)
        chunks.append((n0, ns))
        n0 += ns
    return chunks


def _build_program(k_cap):
    """One-expert MLP over k_cap samples; same program runs SPMD on all 8 cores."""
    chunks = _make_chunks(k_cap)
    KC1, MC1 = LAT_DIM // 128, NB_NEUR // 128  # 4, 8
    KC2, MC2 = NB_NEUR // 128, OUT_DIM // 128  # 8, 4

    nc = bacc.Bacc(
        "TRN2",
        target_bir_lowering=False,
        debug=False,
        enable_asserts=False,
        num_devices=N_CORES,
    )
    epsT = nc.dram_tensor("epsT", [LAT_DIM, k_cap], F32R, kind="ExternalInput").ap()
    w1 = nc.dram_tensor("w1", [LAT_DIM, NB_NEUR], F32R, kind="ExternalInput").ap()
    b1 = nc.dram_tensor("b1", [128, MC1], F32, kind="ExternalInput").ap()
    w2 = nc.dram_tensor("w2", [NB_NEUR, OUT_DIM], F32R, kind="ExternalInput").ap()
    b2 = nc.dram_tensor("b2", [128, MC2], F32, kind="ExternalInput").ap()
    yT = nc.dram_tensor("yT", [OUT_DIM, k_cap], F32, kind="ExternalOutput").ap()

    with tile.TileContext(nc) as tc, ExitStack() as ctx:
        wpool = ctx.enter_context(tc.tile_pool(name="weights", bufs=1))
        xpool = ctx.enter_context(tc.tile_pool(name="x", bufs=2))
        hpool = ctx.enter_context(tc.tile_pool(name="h", bufs=2))
        ypool = ctx.enter_context(tc.tile_pool(name="y", bufs=6))
        p1pool = ctx.enter_context(tc.tile_pool(name="p1", bufs=5, space="PSUM"))
        p2pool = ctx.enter_context(tc.tile_pool(name="p2", bufs=3, space="PSUM"))

        w1t = []
        for kc in range(KC1):
            t = wpool.tile([128, NB_NEUR], F32R, tag=f"w1_{kc}")
            nc.sync.dma_start(t[:], w1[kc * 128 : (kc + 1) * 128, :])
            w1t.append(t)
        w2t = []
        for kc in range(KC2):
            t = wpool.tile([128, OUT_DIM], F32R, tag=f"w2_{kc}")
            nc.sync.dma_start(t[:], w2[kc * 128 : (kc + 1) * 128, :])
            w2t.append(t)
        b1t = wpool.tile([128, MC1], F32, tag="b1")
        nc.sync.dma_start(b1t[:], b1[:])
        b2t = wpool.tile([128, MC2], F32, tag="b2")
        nc.sync.dma_start(b2t[:], b2[:])

        for n0, ns in chunks:
            xt = []
            for kc in range(KC1):
                t = xpool.tile([128, ns], F32R, tag=f"x{kc}")
                nc.sync.dma_start(t[:], epsT[kc * 128 : (kc + 1) * 128, n0 : n0 + ns])
                xt.append(t)

            ht = []
            for mc in range(MC1):
                ps = p1pool.tile([128, ns], F32, tag="p1")
                for kc in range(KC1):
                    nc.tensor.matmul(
                        ps[:],
                        w1t[kc][:, mc * 128 : (mc + 1) * 128],
                        xt[kc][:],
                        start=(kc == 0),
                        stop=(kc == KC1 - 1),
                    )
                h = hpool.tile([128, ns], F32R, tag=f"h{mc}")
                nc.scalar.activation(h[:], ps[:], SIG, bias=b1t[:, mc : mc + 1])
                ht.append(h)

            for oc in range(MC2):
                ps = p2pool.tile([128, ns], F32, tag="p2")
                for kc in range(KC2):
                    nc.tensor.matmul(
                        ps[:],
                        w2t[kc][:, oc * 128 : (oc + 1) * 128],
                        ht[kc][:],
                        start=(kc == 0),
                        stop=(kc == KC2 - 1),
                    )
                y = ypool.tile([128, ns], F32, tag="y")
                nc.vector.tensor_scalar_add(y[:], ps[:], b2t[:, oc : oc + 1])
                nc.sync.dma_start(yT[oc * 128 : (oc + 1) * 128, n0 : n0 + ns], y[:])

    nc.compile()
    return nc


def get_program(k_cap):
    if k_cap not in _program_cache:
        _program_cache[k_cap] = _build_program(k_cap)
    return _program_cache[k_cap]


def _softplus(x):
    x = x.astype(np.float64)
    return (np.maximum(x, 0.0) + np.log1p(np.exp(-np.abs(x)))).astype(np.float32)


def kernel(epsilon, comp_idx, mu, rho, W1, b1, W2, b2, _trace=False):
    epsilon = np.asarray(epsilon, dtype=np.float32)
    comp_idx = np.asarray(comp_idx, dtype=np.int32)
    mu = np.asarray(mu, dtype=np.float32)
    rho = np.asarray(rho, dtype=np.float32)
    W1 = np.asarray(W1, dtype=np.float32)
    b1 = np.asarray(b1, dtype=np.float32)
    W2 = np.asarray(W2, dtype=np.float32)
    b2 = np.asarray(b2, dtype=np.float32)

    n = epsilon.shape[0]
    sigma = _softplus(rho)  # [C]

    sels = [np.nonzero(comp_idx == c)[0] for c in range(NB_COMP)]
    counts = [len(s) for s in sels]
    k_cap = max(256, -(-max(counts) // 256) * 256)

    nc = get_program(k_cap)

    in_maps = []
    for c in range(NB_COMP):
        sel = sels[c]
        epsT = np.zeros((LAT_DIM, k_cap), dtype=np.float32)
        if len(sel):
            epsT[:, : len(sel)] = epsilon[sel].T
        w1p = (W1[c] * sigma[c]).astype(np.float32)
        b1p = (b1[c].astype(np.float64) + mu[c].astype(np.float64) @ W1[c].astype(np.float64)).astype(np.float32)
        in_maps.append(
            {
                "epsT": epsT,
                "w1": np.ascontiguousarray(w1p),
                "b1": np.ascontiguousarray(b1p.reshape(NB_NEUR // 128, 128).T),
                "w2": np.ascontiguousarray(W2[c]),
                "b2": np.ascontiguousarray(b2[c].reshape(OUT_DIM // 128, 128).T),
            }
        )

    res = run_bass_kernel_spmd(
        nc,
        in_maps,
        core_ids=list(range(N_CORES)),
        trace=_trace,
        trace_cores=list(range(N_CORES)) if _trace else None,
    )

    out = np.zeros((n, OUT_DIM), dtype=np.float32)
    for c in range(NB_COMP):
        sel = sels[c]
        if len(sel):
            out[sel] = res.results[c]["yT"][:, : len(sel)].T
    if _trace:
        return out, res
    return out


# revision 5
# speedup vs baseline: 1.0979x; 1.0979x over previous
"""GNN ensemble MoE-routing kernel for Trainium2 (8 NeuronCores).

Reference computes all 8 expert MLPs for every sample then selects one
(8x wasted FLOPs). This kernel routes on the host instead: samples are
gathered per expert, and core c runs ONLY expert c's MLP over the
samples routed to it (expert-parallel sharding).

Math folding (exact):
  lat = eps*sigma_c + mu_c  =>  lat @ W1_c = eps @ (sigma_c*W1_c) + mu_c@W1_c
so the device computes  sigmoid(eps @ W1p + b1p) @ W2 + b2  with
  W1p = sigma_c * W1_c,  b1p = b1_c + mu_c @ W1_c  (folded on host).

Device layout: features on SBUF partitions, samples on the free axis.
  epsT [512, K]  ->  H^T [1024, K]  ->  Y^T [512, K]
Weights load in natural layout as matmul lhsT (out = lhsT.T @ rhs), the
b1 bias+sigmoid fuse into one ScalarE activation (per-partition bias),
b2 is a DVE tensor_scalar add. Matmuls run as float32r (1 cycle/row for
moving dim >= 256, vs 4 cycles/row for plain fp32).
"""

from contextlib import ExitStack

import numpy as np

import concourse.bass as bass
import concourse.tile as tile
from concourse import bacc, mybir
from concourse.bass_utils import run_bass_kernel_spmd

NB_COMP = 8
LAT_DIM = 512
NB_NEUR = 1024
OUT_DIM = 512
N_CORES = 8

F32 = mybir.dt.float32
F32R = mybir.dt.float32r
SIG = mybir.ActivationFunctionType.Sigmoid
N_WARMUP = 24

_program_cache = {}


def _make_chunks(k_cap):
    chunks = []
    n0 = 0
    while n0 < k_cap:
        ns = min(512, k_cap - n0)
        chunks.append((n0, ns))
        n0 += ns
    return chunks


def _build_program(k_cap):
    """One-expert MLP over k_cap samples; same program runs SPMD on all 8 cores."""
    chunks = _make_chunks(k_cap)
    KC1, MC1 = LAT_DIM // 128, NB_NEUR // 128  # 4, 8
    KC2, MC2 = NB_NEUR // 128, OUT_DIM // 128  # 8, 4

    nc = bacc.Bacc(
        "TRN2",
        target_bir_lowering=False,
        debug=False,
        enable_asserts=False,
        num_devices=N_CORES,
    )
    epsT = nc.dram_tensor("epsT", [LAT_DIM, k_cap], F32R, kind="ExternalInput").ap()
    w1 = nc.dram_tensor("w1", [LAT_DIM, NB_NEUR], F32R, kind="ExternalInput").ap()
    b1 = nc.dram_tensor("b1", [128, MC1], F32, kind="ExternalInput").ap()
    w2 = nc.dram_tensor("w2", [NB_NEUR, OUT_DIM], F32R, kind="ExternalInput").ap()
    b2 = nc.dram_tensor("b2", [128, MC2], F32, kind="ExternalInput").ap()
    yT = nc.dram_tensor("yT", [OUT_DIM, k_cap], F32, kind="ExternalOutput").ap()

    with tile.TileContext(nc) as tc, ExitStack() as ctx:
        wpool = ctx.enter_context(tc.tile_pool(name="weights", bufs=1))
        xpool = ctx.enter_context(tc.tile_pool(name="x", bufs=3))
        hpool = ctx.enter_context(tc.tile_pool(name="h", bufs=2))
        ypool = ctx.enter_context(tc.tile_pool(name="y", bufs=6))
        p1pool = ctx.enter_context(tc.tile_pool(name="p1", bufs=5, space="PSUM"))
        p2pool = ctx.enter_context(tc.tile_pool(name="p2", bufs=3, space="PSUM"))

        # PE warm-up: the HAM clock gate releases (1.2 -> 2.4 GHz) after
        # ~3.4us of sustained matmul activity. Burn dummy matmuls on a
        # memset tile while the first input DMAs stream, so real matmuls
        # start at full clock instead of spending their first ~7us cold.
        warm_f = wpool.tile([128, 512], F32, tag="warmf")
        nc.vector.memset(warm_f[:], 1.0)
        warm = wpool.tile([128, 512], F32R, tag="warm")
        nc.scalar.copy(warm[:], warm_f[:])
        wps = p1pool.tile([128, 512], F32, tag="p1")
        for _ in range(N_WARMUP):
            nc.tensor.matmul(wps[:], warm[:, :128], warm[:], start=True, stop=True)

        # DMA issue order matters: HWDGE queues are FIFO, so everything
        # enqueued ahead of the first chunk's inputs delays the first real
        # matmul. Chunk-0 epsT + W1 + b1 go first (mm1 deps), then W2/b2
        # (needed ~14us later), then later chunks.
        first_xt = []
        n0_0, ns_0 = chunks[0]
        for kc in range(KC1):
            t = xpool.tile([128, ns_0], F32R, tag=f"x{kc}")
            nc.sync.dma_start(t[:], epsT[kc * 128 : (kc + 1) * 128, n0_0 : n0_0 + ns_0])
            first_xt.append(t)
        w1t = []
        for kc in range(KC1):
            t = wpool.tile([128, NB_NEUR], F32R, tag=f"w1_{kc}")
            nc.sync.dma_start(t[:], w1[kc * 128 : (kc + 1) * 128, :])
            w1t.append(t)
        b1t = wpool.tile([128, MC1], F32, tag="b1")
        nc.sync.dma_start(b1t[:], b1[:])
        w2t = []
        for kc in range(KC2):
            t = wpool.tile([128, OUT_DIM], F32R, tag=f"w2_{kc}")
            nc.sync.dma_start(t[:], w2[kc * 128 : (kc + 1) * 128, :])
            w2t.append(t)
        b2t = wpool.tile([128, MC2], F32, tag="b2")
        nc.sync.dma_start(b2t[:], b2[:])

        for ci, (n0, ns) in enumerate(chunks):
            if ci == 0:
                xt = first_xt
            else:
                xt = []
                for kc in range(KC1):
                    t = xpool.tile([128, ns], F32R, tag=f"x{kc}")
                    nc.sync.dma_start(
                        t[:], epsT[kc * 128 : (kc + 1) * 128, n0 : n0 + ns]
                    )
                    xt.append(t)

            ht = []
            for mc in range(MC1):
                ps = p1pool.tile([128, ns], F32, tag="p1")
                for kc in range(KC1):
                    nc.tensor.matmul(
                        ps[:],
                        w1t[kc][:, mc * 128 : (mc + 1) * 128],
                        xt[kc][:],
                        start=(kc == 0),
                        stop=(kc == KC1 - 1),
                    )
                h = hpool.tile([128, ns], F32R, tag=f"h{mc}")
                nc.scalar.activation(h[:], ps[:], SIG, bias=b1t[:, mc : mc + 1])
                ht.append(h)

            for oc in range(MC2):
                ps = p2pool.tile([128, ns], F32, tag="p2")
                for kc in range(KC2):
                    nc.tensor.matmul(
                        ps[:],
                        w2t[kc][:, oc * 128 : (oc + 1) * 128],
                        ht[kc][:],
                        start=(kc == 0),
                        stop=(kc == KC2 - 1),
                    )
                y = ypool.tile([128, ns], F32, tag="y")
                nc.vector.tensor_scalar_add(y[:], ps[:], b2t[:, oc : oc + 1])
                nc.sync.dma_start(yT[oc * 128 : (oc + 1) * 128, n0 : n0 + ns], y[:])

    nc.compile()
    return nc


def get_program(k_cap):
    if k_cap not in _program_cache:
        _program_cache[k_cap] = _build_program(k_cap)
    return _program_cache[k_cap]


def _softplus(x):
    x = x.astype(np.float64)
    return (np.maximum(x, 0.0) + np.log1p(np.exp(-np.abs(x)))).astype(np.float32)


def kernel(epsilon, comp_idx, mu, rho, W1, b1, W2, b2, _trace=False):
    epsilon = np.asarray(epsilon, dtype=np.float32)
    comp_idx = np.asarray(comp_idx, dtype=np.int32)
    mu = np.asarray(mu, dtype=np.float32)
    rho = np.asarray(rho, dtype=np.float32)
    W1 = np.asarray(W1, dtype=np.float32)
    b1 = np.asarray(b1, dtype=np.float32)
    W2 = np.asarray(W2, dtype=np.float32)
    b2 = np.asarray(b2, dtype=np.float32)

    n = epsilon.shape[0]
    sigma = _softplus(rho)  # [C]

    sels = [np.nonzero(comp_idx == c)[0] for c in range(NB_COMP)]
    counts = [len(s) for s in sels]
    k_cap = max(256, -(-max(counts) // 256) * 256)

    nc = get_program(k_cap)

    in_maps = []
    for c in range(NB_COMP):
        sel = sels[c]
        epsT = np.zeros((LAT_DIM, k_cap), dtype=np.float32)
        if len(sel):
            epsT[:, : len(sel)] = epsilon[sel].T
        w1p = (W1[c] * sigma[c]).astype(np.float32)
        b1p = (b1[c].astype(np.float64) + mu[c].astype(np.float64) @ W1[c].astype(np.float64)).astype(np.float32)
        in_maps.append(
            {
                "epsT": epsT,
                "w1": np.ascontiguousarray(w1p),
                "b1": np.ascontiguousarray(b1p.reshape(NB_NEUR // 128, 128).T),
                "w2": np.ascontiguousarray(W2[c]),
                "b2": np.ascontiguousarray(b2[c].reshape(OUT_DIM // 128, 128).T),
            }
        )

    res = run_bass_kernel_spmd(
        nc,
        in_maps,
        core_ids=list(range(N_CORES)),
        trace=_trace,
        trace_cores=list(range(N_CORES)) if _trace else None,
    )

    out = np.zeros((n, OUT_DIM), dtype=np.float32)
    for c in range(NB_COMP):
        sel = sels[c]
        if len(sel):
            out[sel] = res.results[c]["yT"][:, : len(sel)].T
    if _trace:
        return out, res
    return out
